# revision 46
# baseline (speedup 1.0000x reference)
"""Multi-head causal attention (B=4, S=2048, D=1024, H=16) on 8 trn2 cores.

Sharding: tensor-parallel over heads x data-parallel over batch.
core c -> (batch b = c//2, head-group hg = c%2 of 8 heads). Every core runs
an identical SPMD program on different data; the host sums the two partial
outputs per batch (the "all-reduce after W_o" done at gather time) and folds
the Wo @ bv + bo constant.

Key design points (vs a 480us f32r baseline):
  - All matmul inputs bf16 (tolerance 2e-2; this lands ~5e-3).
  - Every matmul contracts K=128 so the PE HAM clock-gate never throttles
    (K<128 matmuls don't count as "busy" and drop the PE to 1.2GHz).
    Score matmuls get K=128 via per-head Q tiles zero-padded in the other
    head's 64 feature rows.
  - Weights + K^T/V~/Q/ctx all SBUF-resident; ctx never touches DRAM.
  - QKV weights and per-superblock x slices are packed host-side into
    single wide-line arrays: one 3KB-line DMA descriptor per 128-row chunk
    covers all three projections (descriptor issue time, ~0.6us each, was
    the startup bottleneck).
  - Causal trimming: for diagonal key-block j only query columns >= 128*j
    are computed anywhere (scores/mask/exp/PV). The fine triangular mask
    is applied by accumulating identity.T @ maskbias (0/-30) into the
    score PSUM before exp - no vector-engine mask work.
  - Softmax denominators come free as PV row 64 via a ones column in each
    head's V block (stationary M=65; ones prefilled once at startup). The
    PV accumulator bank is released by a single DVE copy; the normalize
    chain (reciprocal, gpsimd broadcast, multiply) runs entirely off-PSUM
    on otherwise-idle engines. Odd heads' normalized ctx is written with a
    cross-quadrant DVE write (HW-verified) to pack ctx^T for the output
    projection.
  - PV matmul groups trail their score groups by one group, so the PE
    never waits on the scalar engine's exp latency; Q/K projection
    PSUM->SBUF moves ride on the vector engine (tensor_scalar_add with
    bias), keeping the scalar engine exp-only until the final superblock.
"""

import sys

import numpy as np

_BASS_PATH = "/opt/trn_rl_repo"
if _BASS_PATH not in sys.path:
    sys.path.insert(0, _BASS_PATH)

B, S, D, H, DK = 4, 2048, 1024, 16, 64
NCORES = 8
FH = 512  # features per core (8 heads)
HL = 8  # local heads
NSC = 4  # seq superblocks of 512
SQ = 512
NDM = 8  # d_model chunks of 128
NEGB = -30.0  # causal mask bias (exp(-30+s) ~ 0 for |s|<=8)
W3 = 3 * FH  # packed q|k|v width

_cache = {}


def _build():
    import concourse.bacc as bacc
    import concourse.mybir as mybir
    from concourse.tile import TileContext

    f32, bf16 = mybir.dt.float32, mybir.dt.bfloat16
    AF = mybir.ActivationFunctionType

    nc = bacc.Bacc("TRN2", target_bir_lowering=False, debug=False, num_devices=1)

    # packed x: [D, NSC*W3], superblock-major: cols sb*W3+{q:0,k:512,v:1024}
    xall_d = nc.dram_tensor("xall", [D, NSC * W3], bf16, kind="ExternalInput").ap()
    # packed weights^T: [D, q|k|v]
    wqkv_d = nc.dram_tensor("wqkv", [D, W3], bf16, kind="ExternalInput").ap()
    wo_d = nc.dram_tensor("wo", [FH, D], bf16, kind="ExternalInput").ap()
    # packed [ident | maskbias]
    im_d = nc.dram_tensor("im", [128, 256], bf16, kind="ExternalInput").ap()
    # packed per-chunk biases: cols 0..3 = bq chunks, 4..7 = bk chunks
    bqk_d = nc.dram_tensor("bqk", [128, 8], f32, kind="ExternalInput").ap()
    out_d = nc.dram_tensor("out", [S, D], bf16, kind="ExternalOutput").ap()

    with TileContext(nc) as tc:
        with (
            tc.tile_pool(name="res", bufs=1) as res,
            tc.tile_pool(name="st", bufs=1) as st,
            tc.tile_pool(name="psum", bufs=1, space="PSUM") as psp,
        ):
            # ---- resident tiles ----
            kt = [res.tile([128, S], bf16, name=f"kt{i}", tag=f"kt{i}") for i in range(4)]
            # V~ per key block: 8 heads x (64 V cols + ones col), stride 65
            vaug = [
                res.tile([128, 520], bf16, name=f"va{k}", tag=f"va{k}")
                for k in range(16)
            ]
            w_sb = [res.tile([128, W3], bf16, name=f"w{i}", tag=f"w{i}") for i in range(NDM)]
            wo_sb = [
                res.tile([128, D], bf16, name=f"wo{fc}", tag=f"wo{fc}")
                for fc in range(4)
            ]
            im_t = res.tile([128, 256], bf16, name="im", tag="im")
            bqk_t = res.tile([128, 8], f32, name="bqk", tag="bqk")
            # per-head zero-padded Q tiles, double-buffered over superblocks:
            # head h data lives in rows (h%2)*64..+64, other 64 rows stay 0
            qz = [
                [
                    res.tile([128, SQ], bf16, name=f"qz{s}_{h}", tag=f"qz{s}_{h}")
                    for h in range(HL)
                ]
                for s in range(2)
            ]
            # SBUF-resident ctx^T (features x queries), double-buffered
            cfs = [
                [
                    res.tile([128, SQ], bf16, name=f"cf{s}_{fc}", tag=f"cf{s}_{fc}")
                    for fc in range(4)
                ]
                for s in range(2)
            ]

            # one-time fills (cheap, overlap the startup DMAs)
            for s in range(2):
                for h in range(HL):
                    zr = 64 if (h % 2 == 0) else 0
                    nc.vector.memset(qz[s][h][zr : zr + 64, :], 0.0)
            for kb in range(16):
                va3 = vaug[kb][:, 0 : HL * 65].rearrange("p (h e) -> p h e", e=65)
                nc.vector.memset(va3[:, :, 64:65], 1.0)

            def load_x(sc, box, eng=None):
                eng = eng or nc.sync
                xr = []
                for dm in range(NDM):
                    xt = st.tile(
                        [128, W3], bf16, name=f"x{dm}", tag=f"x{dm}", bufs=3
                    )
                    eng.dma_start(
                        xt[:],
                        xall_d[dm * 128 : (dm + 1) * 128, sc * W3 : (sc + 1) * W3],
                    )
                    xr.append(xt)
                box["x"] = xr

            def make_proj_thunks(sc, box=None):
                box = box if box is not None else {}
                thunks = []
                for pname in ("q", "k", "v"):
                    off = {"q": 0, "k": FH, "v": 2 * FH}[pname]

                    for gi in range(4):

                        def group(pname=pname, off=off, gi=gi, box=box, sc=sc):
                            if "x" not in box:
                                load_x(sc, box)
                            xr = box["x"]
                            if pname in ("q", "k"):
                                pp = psp.tile(
                                    [128, SQ], f32, name="pp", tag="pp", bufs=2
                                )
                                for dm in range(NDM):
                                    nc.tensor.matmul(
                                        pp[:],
                                        w_sb[dm][:, off + gi * 128 : off + (gi + 1) * 128],
                                        xr[dm][:, off : off + SQ],
                                        start=(dm == 0),
                                        stop=(dm == NDM - 1),
                                    )
                                if pname == "k":
                                    # gpsimd is mostly idle; keep the DVE
                                    # queue short so cp-releasing copies in
                                    # the attention stream aren't delayed
                                    nc.vector.tensor_scalar_add(
                                        kt[gi][:, sc * SQ : (sc + 1) * SQ],
                                        pp[:],
                                        bqk_t[:, 4 + gi : 5 + gi],
                                    )  # (PSUM source: must stay off gpsimd)
                                else:
                                    s = sc % 2
                                    nc.vector.tensor_scalar_add(
                                        qz[s][2 * gi][0:64, :],
                                        pp[0:64, :],
                                        bqk_t[0:64, gi : gi + 1],
                                    )
                                    nc.vector.tensor_scalar_add(
                                        qz[s][2 * gi + 1][64:128, :],
                                        pp[64:128, :],
                                        bqk_t[64:128, gi : gi + 1],
                                    )
                            else:  # v
                                kb = sc * 4 + gi
                                pp = psp.tile(
                                    [128, FH], f32, name="pp", tag="pp", bufs=2
                                )
                                for dm in range(NDM):
                                    nc.tensor.matmul(
                                        pp[:],
                                        xr[dm][:, off + gi * 128 : off + (gi + 1) * 128],
                                        w_sb[dm][:, off : off + FH],
                                        start=(dm == 0),
                                        stop=(dm == NDM - 1),
                                    )
                                va3 = vaug[kb][:, 0 : HL * 65].rearrange(
                                    "p (h e) -> p h e", e=65
                                )
                                pp3 = pp[:].rearrange("p (h e) -> p h e", e=64)
                                nc.vector.tensor_copy(va3[:, :, 0:64], pp3[:])

                        thunks.append(group)
                return thunks

            def make_attn_stream(h, sb):
                """(score-group, pv-group) thunk pairs for one (head, sb)."""
                ti = h // 2
                nkb = 4 * (sb + 1)
                # off-diagonal blocks first (full N), then diagonal ascending
                kbs = list(range(4 * sb)) + list(range(4 * sb, 4 * sb + 4))
                state = {}

                def sg(b0):
                    group = []
                    for i in range(b0, b0 + 4):
                        kb = kbs[i]
                        j = kb - 4 * sb  # >=0 on diagonal blocks
                        c0 = 128 * j if j >= 0 else 0
                        sp = psp.tile([128, SQ], f32, name="sp", tag="sp", bufs=4)
                        nc.tensor.matmul(
                            sp[:, c0:SQ],
                            kt[ti][:, kb * 128 : (kb + 1) * 128],
                            qz[sb % 2][h][:, c0:SQ],
                            start=True,
                            stop=(j < 0),
                        )
                        if j >= 0:
                            nc.tensor.matmul(
                                sp[:, c0 : c0 + 128],
                                im_t[:, 0:128],
                                im_t[:, 128:256],
                                start=False,
                                stop=True,
                            )
                        es = st.tile([128, SQ], bf16, name="es", tag="es", bufs=8)
                        nc.scalar.activation(es[:, c0:SQ], sp[:, c0:SQ], AF.Exp)
                        group.append((kb, c0, es))
                    state[b0] = group

                def pg(b0):
                    if b0 == 0:
                        state["cp"] = psp.tile(
                            [128, SQ], f32, name="cp", tag="cp", bufs=2
                        )
                        state["emitted"] = 0
                    cp = state["cp"]
                    for kb, c0, es in state.pop(b0):
                        nc.tensor.matmul(
                            cp[0:65, c0:SQ],
                            vaug[kb][:, h * 65 : h * 65 + 65],
                            es[:, c0:SQ],
                            start=(state["emitted"] == 0),
                            stop=(state["emitted"] == nkb - 1),
                        )
                        state["emitted"] += 1
                    if b0 + 4 >= nkb:
                        po = (h % 2) * 64
                        if sb == NSC - 1 and h >= HL - 2:
                            # kernel tail: shorter chain (scalar is idle by
                            # now and nothing else needs the PSUM bank)
                            d1 = st.tile([1, SQ], f32, name="d1", tag="d1", bufs=4)
                            nc.scalar.copy(d1[:], cp[64:65, :])
                            rc1 = st.tile(
                                [1, SQ], f32, name="rc1", tag="rc1", bufs=4
                            )
                            nc.vector.reciprocal_approx_fast(rc1[:], d1[:])
                            rb = st.tile([64, SQ], f32, name="rb", tag="rb", bufs=6)
                            nc.gpsimd.partition_broadcast(rb[:], rc1[:])
                            nc.vector.tensor_mul(
                                cfs[sb % 2][ti][po : po + 64, :], cp[0:64, :], rb[:]
                            )
                            return
                        # single DVE copy releases the PSUM bank; the rest of
                        # the normalize chain runs off SBUF on idle engines.
                        # bf16 keeps the copy short - this is the hottest
                        # spot in the DVE queue (cp reuse waits on it)
                        cu = st.tile([65, SQ], bf16, name="cu", tag="cu", bufs=6)
                        nc.vector.tensor_copy(cu[:], cp[0:65, :])
                        d1 = st.tile([1, SQ], f32, name="d1", tag="d1", bufs=4)
                        nc.vector.tensor_copy(d1[:], cu[64:65, :])
                        rc1 = st.tile([1, SQ], f32, name="rc1", tag="rc1", bufs=4)
                        nc.vector.reciprocal_approx_fast(rc1[:], d1[:])
                        rb = st.tile([64, SQ], f32, name="rb", tag="rb", bufs=6)
                        nc.gpsimd.partition_broadcast(rb[:], rc1[:])
                        nc.vector.tensor_mul(
                            cfs[sb % 2][ti][po : po + 64, :], cu[0:64, :], rb[:]
                        )

                return [
                    (
                        (lambda b0=b0: sg(b0)),
                        (lambda b0=b0: pg(b0)),
                    )
                    for b0 in range(0, nkb, 4)
                ]

            def make_o_thunks(sb):
                thunks = []
                for qb in range(4):
                    for n2 in range(2):

                        def group(qb=qb, n2=n2, sb=sb):
                            cfc = cfs[sb % 2]
                            pp = psp.tile([128, SQ], f32, name="pp", tag="pp", bufs=2)
                            for fc in range(4):
                                nc.tensor.matmul(
                                    pp[:],
                                    cfc[fc][:, qb * 128 : (qb + 1) * 128],
                                    wo_sb[fc][:, n2 * SQ : (n2 + 1) * SQ],
                                    start=(fc == 0),
                                    stop=(fc == 3),
                                )
                            ob = st.tile([128, SQ], bf16, name="ob", tag="ob", bufs=4)
                            nc.vector.tensor_copy(ob[:], pp[:])
                            nc.sync.dma_start(
                                out_d[
                                    sb * SQ + qb * 128 : sb * SQ + (qb + 1) * 128,
                                    n2 * SQ : (n2 + 1) * SQ,
                                ],
                                ob[:],
                            )

                        thunks.append(group)
                return thunks

            # ---- emission schedule ----
            # startup: weights on the sync DMA queue, x chunks on the scalar
            # (HWDGE) queue - two parallel issue streams, few descriptors
            box0 = {}
            load_x(0, box0, eng=nc.scalar)
            for dm in range(NDM):
                nc.sync.dma_start(w_sb[dm][:], wqkv_d[dm * 128 : (dm + 1) * 128, :])
            nc.sync.dma_start(bqk_t[:], bqk_d[:])
            nc.sync.dma_start(im_t[:], im_d[:])
            # prefetch next superblock's x during sb0 (sb0 is too short to
            # hide a 3MB single-queue transfer started lazily)
            box1 = {}
            xr1 = []
            for dm in range(NDM):
                xt = st.tile([128, W3], bf16, name=f"x{dm}", tag=f"x{dm}", bufs=3)
                eng = nc.sync if dm % 2 == 0 else nc.scalar
                eng.dma_start(
                    xt[:], xall_d[dm * 128 : (dm + 1) * 128, W3 : 2 * W3]
                )
                xr1.append(xt)
            box1["x"] = xr1
            box2 = {}
            xr2 = []
            for dm in range(NDM):
                xt = st.tile([128, W3], bf16, name=f"x{dm}", tag=f"x{dm}", bufs=3)
                nc.scalar.dma_start(
                    xt[:], xall_d[dm * 128 : (dm + 1) * 128, 2 * W3 : 3 * W3]
                )
                xr2.append(xt)
            box2["x"] = xr2
            for t in make_proj_thunks(0, box=box0):
                t()
            for fc in range(4):
                nc.sync.dma_start(wo_sb[fc][:], wo_d[fc * 128 : (fc + 1) * 128, :])
            for sb in range(NSC):
                pairs = []
                for h in range(HL):
                    pairs += make_attn_stream(h, sb)
                # flatten with one-group PV lag: SG_g ; PG_{g-1} ; ...
                batches = [pairs[0][0]]
                for g in range(1, len(pairs)):
                    batches.append(pairs[g][0])
                    batches.append(pairs[g - 1][1])
                batches.append(pairs[-1][1])
                warm = []
                if sb < NSC - 1:
                    pbox = box1 if sb == 0 else (box2 if sb == 1 else None)
                    warm += make_proj_thunks(sb + 1, box=pbox)
                if sb >= 1:
                    warm += make_o_thunks(sb - 1)
                # on the last superblock the scalar engine saturates late in
                # the phase; land the spare PE (o-proj) work early instead of
                # spreading it evenly
                nb = len(batches) // 2 if sb == NSC - 1 else len(batches)
                nw = len(warm)
                wi = 0
                for bi, bt in enumerate(batches):
                    bt()
                    while wi < nw and (wi + 1) * nb <= (bi + 1) * nw:
                        warm[wi]()
                        wi += 1
                while wi < nw:
                    warm[wi]()
                    wi += 1
            # final output projection: the fc=3 matmuls wait on the last
            # heads' normalize chains. Pre-open the first four groups on the
            # (now idle) sp banks and run their fc=0..2 matmuls under those
            # chains, so only fc=3 + four full groups remain afterwards.
            cfc3 = cfs[(NSC - 1) % 2]
            ogroups = [(qb, n2) for qb in range(4) for n2 in range(2)]

            def o_final(qb, n2, pp, fc0):
                for fc in range(fc0, 4):
                    nc.tensor.matmul(
                        pp[:],
                        cfc3[fc][:, qb * 128 : (qb + 1) * 128],
                        wo_sb[fc][:, n2 * SQ : (n2 + 1) * SQ],
                        start=(fc == 0),
                        stop=(fc == 3),
                    )
                ob = st.tile([128, SQ], bf16, name="ob", tag="ob", bufs=4)
                nc.vector.tensor_copy(ob[:], pp[:])
                nc.sync.dma_start(
                    out_d[
                        (NSC - 1) * SQ + qb * 128 : (NSC - 1) * SQ + (qb + 1) * 128,
                        n2 * SQ : (n2 + 1) * SQ,
                    ],
                    ob[:],
                )

            opps = []
            for g4 in range(6):
                qb, n2 = ogroups[g4]
                # groups 4/5 ride the cp banks, which free mid-chain (after
                # h6's and h7's normalize multiplies respectively)
                tag = "sp" if g4 < 4 else "cp"
                bufs = 4 if g4 < 4 else 2
                pp = psp.tile([128, SQ], f32, name="op", tag=tag, bufs=bufs)
                for fc in range(3):
                    nc.tensor.matmul(
                        pp[:],
                        cfc3[fc][:, qb * 128 : (qb + 1) * 128],
                        wo_sb[fc][:, n2 * SQ : (n2 + 1) * SQ],
                        start=(fc == 0),
                        stop=False,
                    )
                opps.append(pp)
            for g4 in range(6):
                qb, n2 = ogroups[g4]
                o_final(qb, n2, opps[g4], 3)
            for g4 in range(6, 8):
                qb, n2 = ogroups[g4]
                pp = psp.tile([128, SQ], f32, name="op", tag="sp", bufs=4)
                o_final(qb, n2, pp, 0)

    nc.compile()
    return nc


def kernel(
    q,
    k,
    v,
    mask=None,
    Wq=None,
    bq=None,
    Wk=None,
    bk=None,
    Wv=None,
    bv=None,
    Wo=None,
    bo=None,
    **_unused,
):
    import ml_dtypes

    from concourse.bass_utils import run_bass_kernel_spmd

    if "nc" not in _cache:
        _cache["nc"] = _build()
    nc = _cache["nc"]

    bf16 = ml_dtypes.bfloat16
    q = np.asarray(q, np.float32)
    k = np.asarray(k, np.float32)
    v = np.asarray(v, np.float32)
    Wq = np.asarray(Wq, np.float32)
    Wk = np.asarray(Wk, np.float32)
    Wv = np.asarray(Wv, np.float32)
    Wo = np.asarray(Wo, np.float32)
    bq = np.zeros(D, np.float32) if bq is None else np.asarray(bq, np.float32)
    bk = np.zeros(D, np.float32) if bk is None else np.asarray(bk, np.float32)
    bv = np.zeros(D, np.float32) if bv is None else np.asarray(bv, np.float32)
    bo = np.zeros(D, np.float32) if bo is None else np.asarray(bo, np.float32)

    ident = np.eye(128, dtype=np.float32)
    kk = np.arange(128)[:, None]
    qq = np.arange(128)[None, :]
    maskb = np.where(kk <= qq, 0.0, NEGB)
    im = np.concatenate([ident, maskb], axis=1).astype(bf16)

    # packed x per batch: [D, NSC, 3, SQ] -> [D, NSC*W3]
    xpack = {}
    for b in range(B):
        xp = np.empty((D, NSC, 3, SQ), np.float32)
        for i, arr in enumerate((q, k, v)):
            aT = arr[b].T  # [D, S]
            xp[:, :, i, :] = aT.reshape(D, NSC, SQ)
        xpack[b] = np.ascontiguousarray(xp.reshape(D, NSC * W3)).astype(bf16)

    wqkvs, wos, bqks = {}, {}, {}
    for hg in range(2):
        sl = slice(hg * FH, (hg + 1) * FH)
        wq_p = np.ascontiguousarray(Wq[sl, :].T) * np.float32(0.125)
        wk_p = np.ascontiguousarray(Wk[sl, :].T)
        wv_p = np.ascontiguousarray(Wv[sl, :].T)
        wqkvs[hg] = np.concatenate([wq_p, wk_p, wv_p], axis=1).astype(bf16)
        wos[hg] = np.ascontiguousarray(Wo[:, sl].T).astype(bf16)
        bb = np.empty((128, 8), np.float32)
        for gi in range(4):
            bb[:, gi] = bq[hg * FH + gi * 128 : hg * FH + (gi + 1) * 128] * 0.125
            bb[:, 4 + gi] = bk[hg * FH + gi * 128 : hg * FH + (gi + 1) * 128]
        bqks[hg] = bb

    in_maps = []
    for c in range(NCORES):
        b, hg = c // 2, c % 2
        in_maps.append(
            {
                "xall": xpack[b],
                "wqkv": wqkvs[hg],
                "wo": wos[hg],
                "im": im,
                "bqk": bqks[hg],
            }
        )

    res = run_bass_kernel_spmd(nc, in_maps, list(range(NCORES)))
    out = np.empty((B, S, D), np.float32)
    for b in range(B):
        out[b] = res.results[2 * b]["out"].astype(np.float32) + res.results[
            2 * b + 1
        ]["out"].astype(np.float32)
    const = Wo @ bv + bo  # bv/bo contribution (folds exactly through softmax)
    if np.any(const):
        out += const[None, None, :]
    return out
